# revision 87
# baseline (speedup 1.0000x reference)
"""MHSA Trainium2 Bass kernel (8 NeuronCores, SPMD).

Sharding: core c handles batch b=c//4, head group hg=c%4 (4 of 16 heads).

Key restructurings vs a direct port:
- softmax(sigmoid(s)-0.5) = softmax(0.5*tanh(s/2)) (shift invariance); the
  softmax weight w = e^{t/2}, t=tanh(s/2) in [-1,1], is a minimax quadratic
  c0 + c1*t + c2*t^2: one ACT tanh pass + two DVE ops per score tile, with
  c0 folded into the PV matmul as a rank-1 correction (all-c0 stationary x
  kb-summed folded V) so no engine ever touches it elementwise.
- Wo is folded into the V projection on the host (Wv@Wo, bv@Wo per head), so
  PV directly produces the per-head output [q, 64] plus a softmax-denominator
  column (ones column in the folded-V tile); no on-device output projection.
  Host divides by the denominator and sums heads.
- Scores are built transposed [k, q]; PV uses the score tile as matmul
  stationary so the output lands [q-partition, d-free] (full 128-row output
  partition -> half the PE rows of a [65, q] layout).
- q/k head tensors are stored fp8(e4m3) and QK runs in DoubleRow perf mode
  (0.5 PE cycles/row; the second DoubleRow plane is zeros since head base
  partitions are restricted to 0/64). fp8 score noise washes out in the
  softmax: 1.6e-3 end-to-end vs 1.25e-3 all-bf16.
- q/k projection biases are applied by DVE during PSUM->SBUF evacuation
  (per-partition tensor_scalar), not by PE. Pool (gpsimd) cannot access PSUM
  on TRN2, so it only handles SBUF-side work (memsets, folded-V kb-sums).
- Emission interleaves projection/V-work into the QK/tanh slot stream in
  sub-microsecond pieces (own PSUM bank) so the in-order PE queue never
  blocks the next score tile for long.

Engine budget per core (CoreSim cost model): ACT 134.3us (tanh over all
16.8M scores -- the bottleneck), DVE 132.8us (quadratic + evacuations),
PE 108.3us, Pool 10.4us; span 168.5us vs 300.5us for the previous kernel.
"""
import numpy as np
import ml_dtypes
from contextlib import ExitStack

import concourse.bass as bass
import concourse.tile as tile
from concourse import bacc, mybir
from concourse.bass_utils import run_bass_kernel_spmd

BF = ml_dtypes.bfloat16
F32 = mybir.dt.float32
BF16 = mybir.dt.bfloat16
FP8 = mybir.dt.float8e4

S = 2048          # sequence length
IN = 1024         # input dim
HL = 4            # heads per core
KS = IN // 128    # k subtiles of input dim

# minimax quadratic for e^(t/2) on t in [-1, 1] (relative error ~5e-3)
C0 = 1.0018833788815076
C1 = 0.5152824609958111
C2 = 0.12305588825714274

# pv accumulator banking: 16 query sub-blocks over 3 PSUM banks
PVQ = (7, 7, 2)
PV_START = (0, 7, 14)
PV_STOP = (6, 13, 15)

_CACHE = {}


def _build():
    nc = bacc.Bacc("TRN2", target_bir_lowering=False, debug=False, num_devices=8)
    AF = mybir.ActivationFunctionType
    ALU = mybir.AluOpType

    # x and w are staged in DRAM as [partition, ks, cols] so one DMA covers
    # many input-dim subtiles with 512B+ contiguous runs (SP issue is ~500ns
    # per dma_start -- fewer, bigger DMAs)
    d_x = [nc.dram_tensor(n, [128, KS, S], BF16, kind="ExternalInput")
           for n in ("xq", "xk", "xv")]
    d_w = [nc.dram_tensor(n, [128, KS, 256], BF16, kind="ExternalInput")
           for n in ("wq", "wk", "wvo")]
    d_bq = nc.dram_tensor("bqr", [128, 2], F32, kind="ExternalInput")
    d_bk = nc.dram_tensor("bkr", [128, 2], F32, kind="ExternalInput")
    d_bv = nc.dram_tensor("bvo", [1, 256], BF16, kind="ExternalInput")
    d_out = nc.dram_tensor("out", [HL, 128, 1040], F32, kind="ExternalOutput")

    with tile.TileContext(nc) as tc, ExitStack() as ctx:
        const = ctx.enter_context(tc.tile_pool(name="const", bufs=1))
        persist = ctx.enter_context(tc.tile_pool(name="persist", bufs=1))
        xp = ctx.enter_context(tc.tile_pool(name="xp", bufs=2))
        xvp = ctx.enter_context(tc.tile_pool(name="xvp", bufs=2))
        tp = ctx.enter_context(tc.tile_pool(name="tp", bufs=7))
        up = ctx.enter_context(tc.tile_pool(name="up", bufs=1))
        wp = ctx.enter_context(tc.tile_pool(name="wp", bufs=9))
        pxp = ctx.enter_context(tc.tile_pool(name="pxp", bufs=3))
        qkps = ctx.enter_context(tc.tile_pool(name="qkps", bufs=2, space="PSUM"))
        filps = ctx.enter_context(tc.tile_pool(name="filps", bufs=1, space="PSUM"))
        pvps = ctx.enter_context(tc.tile_pool(name="pvps", bufs=3, space="PSUM"))

        # ---- constant tiles ----
        bq_sb = const.tile([128, 2], F32, tag="bq")
        nc.sync.dma_start(bq_sb[:], d_bq.ap())
        bk_sb = const.tile([128, 2], F32, tag="bk")
        nc.sync.dma_start(bk_sb[:], d_bk.ap())
        bv_sb = const.tile([1, 256], BF16, tag="bv")
        nc.sync.dma_start(bv_sb[:], d_bv.ap())
        ones128 = const.tile([1, 128], BF16, tag="o128")
        nc.gpsimd.memset(ones128[:], 1.0)
        c0c = const.tile([128, 128], F32, tag="c0c")
        nc.gpsimd.memset(c0c[:], C0)
        scratch = const.tile([1, 8], BF16, tag="scr")
        nc.gpsimd.memset(scratch[:], 0.0)
        # preload the tanh ACT table off the critical path
        nc.scalar.activation(scratch[:], scratch[:], AF.Tanh, bias=0.0, scale=1.0)

        w_sb = []
        for i in range(3):
            w_sb.append(const.tile([128, KS, 256], BF16, tag=f"w{i}",
                                   name=f"w_sb{i}"))

        def load_w(i):
            nc.sync.dma_start(w_sb[i][:], d_w[i].ap())

        # ---- persistent activations ----
        qhT = persist.tile([128, 2, 2, S], FP8, tag="qhT")  # [d', mb, i, q]
        khT = persist.tile([128, 2, 2, S], FP8, tag="khT")
        # (zero-plane memsets are emitted mid-Phase-A below so Pool does them
        # while PE chews the first projections)
        vwox = persist.tile([128, 16, HL, 65], BF16, tag="vwox")  # V@Wo | ones
        nc.gpsimd.memset(vwox[:, :, :, 64:65], 1.0)
        vs = [persist.tile([128, HL, 65], F32, tag=f"vs{i}", name=f"vs{i}")
              for i in range(2)]

        x_sb = [xp.tile([128, KS, S], BF16, tag="x", name=f"x_{n}")
                for n in "qk"]
        xv_sb = {}

        def load_x(ti, qc):
            for hf in range(2):
                nc.sync.dma_start(
                    x_sb[ti][:, hf * 4:(hf + 1) * 4, qc * 512:(qc + 1) * 512],
                    d_x[ti].ap()[:, hf * 4:(hf + 1) * 4, qc * 512:(qc + 1) * 512])

        def load_xv(p):
            t = xvp.tile([128, KS, 256], BF16, tag="xv", name=f"xv_{p}")
            xv_sb[p] = t
            nc.sync.dma_start(t[:], d_x[2].ap()[:, :, p * 256:(p + 1) * 256])

        # DMA order (SP queue = emission order): k first so the k projection
        # finishes inside the longer q-load window. Later xv pieces are
        # emitted in fill slots, after the vp reads that free their buffer.
        load_w(1)
        load_x(1, 0)
        load_w(0)
        load_x(0, 0); load_x(0, 1)
        load_x(0, 2); load_x(0, 3)
        load_x(1, 1)
        load_w(2)
        load_xv(0); load_xv(1)
        load_x(1, 2)
        load_x(1, 3)

        # ---- split emission units (each piece <= ~1us of PE) ----
        def qk_proj(ti, dest, bias_sb, qc, mb, pieces=2, evac="dve"):
            # bias is added during the PSUM->SBUF(fp8) evacuation on DVE
            # (Pool cannot read PSUM on real hardware; ACT Identity-evac was
            # tried and couples the tanh stream to projection timing).
            st = {}
            splits = (0, 4, KS) if pieces == 2 else (0, 3, 6, KS)

            def piece(pi):
                def f():
                    if pi == 0:
                        st["ps"] = filps.tile([128, 512], F32, tag="fil",
                                              name=f"pj{ti}_{qc}_{mb}")
                    for ks in range(splits[pi], splits[pi + 1]):
                        nc.tensor.matmul(
                            st["ps"][:],
                            w_sb[ti][:, ks, mb * 128:(mb + 1) * 128],
                            x_sb[ti][:, ks, qc * 512:(qc + 1) * 512],
                            start=(ks == 0), stop=(ks == KS - 1))
                    if splits[pi + 1] == KS:
                        out = dest[:, mb, 0, qc * 512:(qc + 1) * 512]
                        if evac == "act":
                            nc.scalar.activation(out, st["ps"][:], AF.Identity,
                                                 bias=bias_sb[:, mb:mb + 1],
                                                 scale=1.0)
                        else:
                            nc.vector.tensor_scalar(out, st["ps"][:],
                                                    bias_sb[:, mb:mb + 1],
                                                    None, ALU.add)
                return f

            return [piece(i) for i in range(len(splits) - 1)]

        vp_done = set()

        def v_proj(kb):
            st = {}

            def a():
                st["ps"] = filps.tile([128, 512], F32, tag="fil",
                                      name=f"vp{kb}")
                xv_t = xv_sb[kb // 2]
                for ks in range(4):
                    nc.tensor.matmul(st["ps"][:, 0:256],
                                     xv_t[:, ks, (kb % 2) * 128:(kb % 2) * 128 + 128],
                                     w_sb[2][:, ks, :], start=(ks == 0), stop=False)

            def b(kb=kb):
                xv_t = xv_sb[kb // 2]
                for ks in range(4, KS):
                    nc.tensor.matmul(st["ps"][:, 0:256],
                                     xv_t[:, ks, (kb % 2) * 128:(kb % 2) * 128 + 128],
                                     w_sb[2][:, ks, :], start=False, stop=False)
                nc.tensor.matmul(st["ps"][:, 0:256], ones128[:], bv_sb[:],
                                 start=False, stop=True)
                # DVE evacuates (Pool cannot read PSUM on real hardware; an
                # ACT copy would couple the tanh stream to vp/DMA timing)
                nc.vector.tensor_copy(vwox[:, kb, :, 0:64], st["ps"][:, 0:256])
                vp_done.add(kb)
                if kb >= 1:
                    vs_add(kb)

            return [a, b]

        vs_state = [None]

        def vs_add(kb):
            # vs += vwox[:, kb]  (f32 ping-pong accumulation on idle Pool)
            if kb == 1:
                nc.gpsimd.tensor_tensor(vs[0][:], vwox[:, 0, :, :],
                                        vwox[:, 1, :, :], ALU.add)
                vs_state[0] = vs[0]
            else:
                cr = vs_state[0]
                nxt = vs[1] if cr is vs[0] else vs[0]
                nc.gpsimd.tensor_tensor(nxt[:], cr[:], vwox[:, kb, :, :], ALU.add)
                vs_state[0] = nxt

        pv_t = {}     # h -> [3 psum tiles]
        w_tiles = {}  # (h, kb) -> sbuf tile

        def qk_chunk(h, kb, c, t_t, split=False):
            # split=True emits tanh per 512-query half so the half whose
            # qhT data is ready first is not blocked by the other's
            # projection (+186ns ACT init, used where evac arrival staggers)
            pr, off = h // 2, 64 * (h % 2)
            ps = qkps.tile([128, 1024], F32, tag="qk", name=f"qk{h}_{kb}_{c}")
            for i in range(2):
                nc.tensor.matmul(
                    ps[:, i * 512:(i + 1) * 512],
                    khT[off:off + 64, pr, :, kb * 128:(kb + 1) * 128],
                    qhT[off:off + 64, pr, :,
                        c * 1024 + i * 512:c * 1024 + (i + 1) * 512],
                    start=True, stop=True,
                    perf_mode=mybir.MatmulPerfMode.DoubleRow)
                if split:
                    nc.scalar.activation(
                        t_t[:, c * 1024 + i * 512:c * 1024 + (i + 1) * 512],
                        ps[:, i * 512:(i + 1) * 512],
                        AF.Tanh, bias=0.0, scale=1.0 / 16.0)
            if not split:
                nc.scalar.activation(t_t[:, c * 1024:(c + 1) * 1024], ps[:],
                                     AF.Tanh, bias=0.0, scale=1.0 / 16.0)

        def exp_kb(h, kb, t_t):
            # full-kb DVE quadratic (one ts at 4x + one tt at 2x over 2048)
            u = up.tile([128, S], BF16, tag="u", name=f"u{h}_{kb}")
            nc.vector.tensor_scalar(u[:], t_t[:], C2, C1, ALU.mult, ALU.add)
            nc.vector.tensor_tensor(w_tiles[(h, kb)][:], u[:], t_t[:], ALU.mult)

        def pv_unit(h, kb, c):
            w_t = w_tiles[(h, kb)]
            for g2 in range(2):
                for j in range(4):
                    qb = (c * 2 + g2) * 4 + j
                    # start zeroes a whole 2KB zero region (one pv bank)
                    nc.tensor.matmul(
                        pv_t[h][qb // 7][:, qb % 7, :],
                        w_t[:, c * 1024 + g2 * 512 + j * 128:
                            c * 1024 + g2 * 512 + (j + 1) * 128],
                        vwox[:, kb, h, :],
                        start=(kb == 0 and qb in PV_START), stop=False)

        hf_done = set()

        def hf_piece(h, bk_i, final=False):
            # c0-fold rank-1 correction closes this pv bank's accumulation
            # group; evacuate + DMA the bank. The final head evacuates on
            # ACT (idle after the last tanh) instead of the busy DVE.
            def f():
                if bk_i == 0 and not final:
                    flush_pv(h)
                vsf = vs_state[0]
                q0 = sum(PVQ[:bk_i])
                n = PVQ[bk_i]
                for qb in range(q0, q0 + n):
                    nc.tensor.matmul(pv_t[h][bk_i][:, qb - q0, :], c0c[:],
                                     vsf[:, h, :], start=False,
                                     stop=(qb in PV_STOP))
                px = pxp.tile([128, 7, 65], F32, tag="px", name=f"px{h}_{bk_i}")
                if final:
                    nc.scalar.activation(px[:, 0:n, :],
                                         pv_t[h][bk_i][:, 0:n, :],
                                         AF.Copy, bias=0.0, scale=1.0)
                else:
                    nc.vector.tensor_copy(px[:, 0:n, :],
                                          pv_t[h][bk_i][:, 0:n, :])
                nc.sync.dma_start(d_out.ap()[h, :, q0 * 65:(q0 + n) * 65],
                                  px[:, 0:n, :])
                if bk_i == 2:
                    hf_done.add(h)
            return f

        # ---- phase A prologue (k first; Pool zeroes the DoubleRow second
        # planes while PE chews the projections) ----
        nc.gpsimd.memset(khT[:, :, 1, :], 0.0)
        nc.gpsimd.memset(qhT[:, :, 1, :], 0.0)
        for f in (qk_proj(1, khT, bk_sb, 0, 0, evac="dve")
                  + qk_proj(0, qhT, bq_sb, 0, 0, evac="dve")
                  + qk_proj(0, qhT, bq_sb, 1, 0, evac="dve")):
            f()

        # fill schedule: (head, slot) -> emission pieces
        fills = {}

        def put(h, s, *fs):
            fills.setdefault((h, s), []).extend(fs)

        # head 0: mb0 remainder + all 16 v projections. Pieces placed so (a)
        # writers are emitted before their first reader's slot, (b) each slot
        # gets at most ~1us of extra PE work, (c) vp pieces trail the xv DMA.
        q2 = qk_proj(0, qhT, bq_sb, 2, 0, evac="dve")
        q3 = qk_proj(0, qhT, bq_sb, 3, 0, evac="dve")
        k1 = qk_proj(1, khT, bk_sb, 1, 0, evac="dve")
        k2 = qk_proj(1, khT, bk_sb, 2, 0, evac="dve")
        k3 = qk_proj(1, khT, bk_sb, 3, 0, evac="dve")
        vpu = {kb: v_proj(kb) for kb in range(16)}
        put(0, 2, q2[0]); put(0, 3, q2[1], q3[0]); put(0, 4, q3[1])
        put(0, 7, k1[0]); put(0, 8, k1[1])
        put(0, 9, vpu[0][0]); put(0, 10, vpu[0][1], vpu[1][0])
        put(0, 11, vpu[1][1], lambda: load_xv(2))
        put(0, 12, vpu[2][0]); put(0, 13, vpu[2][1], vpu[3][0])
        put(0, 14, vpu[3][1], lambda: load_xv(3))
        put(0, 15, k2[0]); put(0, 16, k2[1])
        put(0, 17, vpu[4][0]); put(0, 18, vpu[4][1], vpu[5][0])
        put(0, 19, vpu[5][1], lambda: load_xv(4))
        put(0, 20, vpu[6][0]); put(0, 21, vpu[6][1], vpu[7][0])
        put(0, 22, vpu[7][1], lambda: load_xv(5), k3[0])
        put(0, 24, k3[1])
        put(0, 25, vpu[8][0]); put(0, 26, vpu[8][1], vpu[9][0])
        put(0, 27, vpu[9][1], lambda: load_xv(6))
        put(0, 28, vpu[10][0]); put(0, 29, vpu[10][1], vpu[11][0])
        put(0, 30, vpu[11][1], lambda: load_xv(7))
        put(0, 31, vpu[12][0])
        # head 1: late v projections, head-0 finish, mb1 projections.
        # km1 qc2/qc3 ride head 2's staggered deadlines (khT kb-block g is
        # first read at h2 slot 8g).
        put(1, 0, vpu[12][1])
        put(1, 1, vpu[13][0])
        put(1, 2, vpu[13][1])
        put(1, 3, vpu[14][0])
        put(1, 4, vpu[14][1])
        put(1, 5, vpu[15][0])
        put(1, 6, vpu[15][1])
        put(1, 8, hf_piece(0, 0))
        put(1, 9, hf_piece(0, 1))
        put(1, 10, hf_piece(0, 2))
        qm1 = [qk_proj(0, qhT, bq_sb, qc, 1, pieces=3, evac="dve")
               for qc in range(4)]
        km1 = [qk_proj(1, khT, bk_sb, qc, 1, pieces=3, evac="dve")
               for qc in range(4)]
        for i in range(4):
            for pi in range(3):
                put(1, 12 + 3 * i + pi, qm1[i][pi])
        for pi in range(3):
            put(1, 24 + pi, km1[0][pi])
            put(1, 28 + pi, km1[1][pi])
            put(2, 2 + pi, km1[2][pi])
            put(2, 10 + pi, km1[3][pi])
        put(2, 5, hf_piece(1, 0))
        put(2, 6, hf_piece(1, 1))
        put(2, 7, hf_piece(1, 2))
        put(3, 4, hf_piece(2, 0))
        put(3, 5, hf_piece(2, 1))
        put(3, 6, hf_piece(2, 2))

        def chunk_order(h):
            if h == 0:
                return [(kb, c) for kq in range(4) for c in range(2)
                        for kb in range(4 * kq, 4 * kq + 4)]
            return [(kb, c) for kb in range(16) for c in range(2)]

        pv_pending = []   # (h, kb, c, slot-appended)
        exp_done = set()
        cur = [0, 0]

        def _pv_front_ready(lagged):
            h, kb, c, es = pv_pending[0]
            if kb not in vp_done or (h, kb, c) not in exp_done:
                return False
            if h > 0 and (h - 1) not in hf_done:
                return False
            if lagged and h == cur[0] and cur[1] - es < LAGV[0]:
                return False
            return True

        def _pv_emit_front():
            h, kb, c, es = pv_pending.pop(0)
            if h not in pv_t:
                pv_t[h] = [pvps.tile([128, 7, 65], F32, tag="pv",
                                     name=f"pv{h}_{g}",
                                     padded_shape=[128, 7, 73])
                           for g in range(3)]
            pv_unit(h, kb, c)

        def drain_pv(budget):
            n = 0
            while pv_pending and n < budget and _pv_front_ready(True):
                _pv_emit_front()
                n += 1

        def flush_pv(h):
            while pv_pending and pv_pending[0][0] <= h:
                assert _pv_front_ready(False), "flush_pv blocked"
                _pv_emit_front()

        LAGV = [4]
        for h in range(HL):
            t_cur = {}
            t_seen = {}
            cur[0] = h
            for slot, (kb, c) in enumerate(chunk_order(h)):
                cur[1] = slot
                if h == HL - 1 and slot >= 26:
                    LAGV[0] = 1  # shrink the final drain tail
                if (h, kb) not in w_tiles:
                    w_tiles[(h, kb)] = wp.tile([128, S], BF16, tag="w",
                                               name=f"w{h}_{kb}")
                if kb not in t_cur:
                    t_cur[kb] = tp.tile([128, S], BF16, tag="t", name=f"t{h}_{kb}")
                for f in fills.get((h, slot), []):
                    f()
                qk_chunk(h, kb, c, t_cur[kb])
                seen = t_seen.setdefault(kb, set())
                seen.add(c)
                if h == HL - 1 and kb == 15:
                    # split the very last exp so only a half-chunk quadratic
                    # remains after the final tanh
                    u = up.tile([128, S], BF16, tag="u", name=f"uf_{c}")
                    nc.vector.tensor_scalar(
                        u[:, 0:1024], t_cur[kb][:, c * 1024:(c + 1) * 1024],
                        C2, C1, ALU.mult, ALU.add)
                    nc.vector.tensor_tensor(
                        w_tiles[(h, kb)][:, c * 1024:(c + 1) * 1024],
                        u[:, 0:1024], t_cur[kb][:, c * 1024:(c + 1) * 1024],
                        ALU.mult)
                    exp_done.add((h, kb, c))
                elif len(seen) == 2:  # both halves scored -> exp whole kb
                    exp_kb(h, kb, t_cur[kb])
                    exp_done.add((h, kb, 0))
                    exp_done.add((h, kb, 1))
                pv_pending.append((h, kb, c, slot))
                drain_pv(2)
        # final head: drain the first-half (c0) PV units and close bank A so
        # its c0/evac/DMA overlaps the last exp and PV work of bank B/C
        while (pv_pending and pv_pending[0][2] == 0
               and _pv_front_ready(False)):
            _pv_emit_front()
        hf_piece(HL - 1, 0, final=True)()
        flush_pv(HL - 1)
        hf_piece(HL - 1, 1, final=True)()
        hf_piece(HL - 1, 2, final=True)()
        assert not pv_pending, "undrained PV work at end"

    nc.compile()
    return nc


def get_module():
    if "nc" not in _CACHE:
        _CACHE["nc"] = _build()
    return _CACHE["nc"]


def make_in_maps(q, k, v, Wq, bq, Wk, bk, Wv, bv, Wo, bo):
    in_maps = []
    for c in range(8):
        b, hg = c // 4, c % 4
        sl = slice(256 * hg, 256 * (hg + 1))
        wvo = np.zeros((IN, 256), np.float64)
        bvo = np.zeros((256,), np.float64)
        for i in range(HL):
            gh = hg * HL + i
            wvo[:, 64 * i:64 * (i + 1)] = (
                Wv[:, 64 * gh:64 * (gh + 1)].astype(np.float64)
                @ Wo[64 * gh:64 * (gh + 1), :].astype(np.float64))
            bvo[64 * i:64 * (i + 1)] = (
                bv[64 * gh:64 * (gh + 1)].astype(np.float64)
                @ Wo[64 * gh:64 * (gh + 1), :].astype(np.float64))

        def pks(a):  # [1024, C] -> [128, 8, C] with row ks*128+p -> [p, ks]
            return np.ascontiguousarray(
                a.reshape(KS, 128, a.shape[1]).transpose(1, 0, 2)).astype(BF)

        in_maps.append({
            "xq": pks(q[b].T), "xk": pks(k[b].T), "xv": pks(v[b].T),
            "wq": pks(Wq[:, sl]), "wk": pks(Wk[:, sl]),
            "wvo": pks(wvo.astype(np.float32)),
            "bqr": np.ascontiguousarray(
                bq[sl].reshape(2, 128).T).astype(np.float32),
            "bkr": np.ascontiguousarray(
                bk[sl].reshape(2, 128).T).astype(np.float32),
            "bvo": bvo.reshape(1, 256).astype(BF),
        })
    return in_maps


def assemble(results, bo):
    out = np.zeros((2, S, 64), np.float32)
    for c in range(8):
        y = np.asarray(results[c]["out"], np.float32).reshape(HL, 128, 16, 65)
        r = y[..., 0:64] / y[..., 64:65]
        # q = (qb//4)*512 + (qb%4)*128 + p
        r = r.transpose(0, 2, 1, 3).reshape(HL, S, 64)
        out[c // 4] += r.sum(axis=0)
    out += np.asarray(bo, np.float32)[None, None, :]
    return out


def kernel(q, k, v, Wq, bq, Wk, bk, Wv, bv, Wo, bo):
    nc = get_module()
    in_maps = make_in_maps(q, k, v, Wq, bq, Wk, bk, Wv, bv, Wo, bo)
    res = run_bass_kernel_spmd(nc, in_maps, core_ids=list(range(8)))
    return assemble(res.results, bo)
